# revision 3
# baseline (speedup 1.0000x reference)
"""Trainium2 Bass kernel for nn_GroupProjection (gnn_message_passing).

Reference computation (B=8, N=8192, D=512, P=4, G=512, GS=16, 3 iters):
    for ite in range(3):
        delta = 0
        for i in range(P):
            gx = upd[:, groups[i], :]                 # gather
            dx = (1/(ite+1)) * gx @ W[i]              # GEMM
            delta[:, groups[i].ravel(), :] += dx      # scatter-add
        upd = upd + delta

Key identity: the gather index equals the scatter index, so
    delta[b, n, :] = (1/(ite+1)) * sum_i count_i[n] * (upd[b, n, :] @ W[i])
where count_i[n] = multiplicity of n in groups[i]. The gather/scatter
disappears into dense GEMMs plus a per-row weighted combine; counts are
computed on host with np.bincount (groups is only 32K ints).

Row n's update depends only on row n, so the whole 3-iteration computation
streams independently over 64 row-tiles of 128 per core, data-parallel over
the batch axis (1 batch element per NeuronCore).

Device layout per row-tile (n = 128 rows):
  updT  [d=4x128 part, n=128]  (x arrives host-transposed, so DMA loads it directly)
  per iter:  Y_i[n,128 x e,512] = sum_k matmul(lhsT=updT[k], rhs=W[i][k])   (float32r)
             delta[n,d] = sum_i (count_i*scale)[n] * Y_i      (DVE per-partition scalars)
             updT += transpose(delta)                          (PE transpose + DVE add)
  final out tile = transpose(updT) + delta -> DMA out.
"""

import numpy as np

B, N, D = 8, 8192, 512
P = 4
NUM_ITER = 3
TP = 128                 # rows per tile
NT = N // TP             # 64 row tiles
KC = D // 128            # 4 contraction chunks
NCORES = 8

_CACHE = {}


def _build():
    import concourse.bass as bass
    import concourse.tile as tile
    from concourse import bacc, mybir

    f32 = mybir.dt.float32
    f32r = mybir.dt.float32r
    MULT = mybir.AluOpType.mult
    ADD = mybir.AluOpType.add

    nc = bacc.Bacc("TRN2", target_bir_lowering=False, debug=False,
                   num_devices=NCORES)

    xT_d = nc.dram_tensor("xT", [D, N], f32, kind="ExternalInput")
    w_d = nc.dram_tensor("w", [P, D, D], f32, kind="ExternalInput")
    c_d = nc.dram_tensor("cnts", [128, NUM_ITER * NT * P], f32,
                         kind="ExternalInput")
    out_d = nc.dram_tensor("out", [N, D], f32, kind="ExternalOutput")
    ident_d = nc.inline_tensor(np.eye(128, dtype=np.float32), "ident")

    with tile.TileContext(nc) as tc:
        with (
            tc.tile_pool(name="const", bufs=1) as constp,
            tc.tile_pool(name="xstage", bufs=4) as xstagep,
            tc.tile_pool(name="updT", bufs=4) as updTp,
            tc.tile_pool(name="delta", bufs=4) as deltap,
            tc.tile_pool(name="outp", bufs=3) as outp,
            tc.tile_pool(name="psumY", bufs=6, space=bass.MemorySpace.PSUM) as psumY,
            tc.tile_pool(name="psumT", bufs=2, space=bass.MemorySpace.PSUM) as psumT,
        ):
            # W: chunk (i, k) lives at columns [(i*KC+k)*D : +D]. The f32r
            # matmul requires operands rounded to f32r by their producer, so
            # DMA into an f32 staging tile and cast-copy on DVE.
            w_stage = constp.tile([128, P * KC * D], f32)
            for i in range(P):
                for k in range(KC):
                    nc.sync.dma_start(
                        w_stage[:, (i * KC + k) * D:(i * KC + k + 1) * D],
                        w_d[i, k * 128:(k + 1) * 128, :])
            w_sb = constp.tile([128, P * KC * D], f32r)
            nc.vector.tensor_copy(w_sb[:], w_stage[:])
            cnt_sb = constp.tile([128, NUM_ITER * NT * P], f32)
            nc.sync.dma_start(cnt_sb[:], c_d[:])
            ident_sb = constp.tile([128, 128], f32)
            nc.sync.dma_start(ident_sb[:], ident_d.ap())

            for t in range(NT):
                x_stage = xstagep.tile([128, KC * 128], f32, tag="xstage")
                for k in range(KC):
                    nc.sync.dma_start(
                        x_stage[:, k * 128:(k + 1) * 128],
                        xT_d[k * 128:(k + 1) * 128, t * TP:(t + 1) * TP])
                updT_t = updTp.tile([128, KC * 128], f32r, tag="updT")
                nc.vector.tensor_copy(updT_t[:], x_stage[:])
                for ite in range(NUM_ITER):
                    ys = []
                    for i in range(P):
                        y = psumY.tile([128, D], f32, tag="y")
                        for k in range(KC):
                            nc.tensor.matmul(
                                y[:],
                                updT_t[:, k * 128:(k + 1) * 128],
                                w_sb[:, (i * KC + k) * D:(i * KC + k + 1) * D],
                                start=(k == 0), stop=(k == KC - 1))
                        ys.append(y)
                    # delta = sum_i (cnt_i / (ite+1)) * Y_i  (scale folded into cnts)
                    delta_t = deltap.tile([128, D], f32, tag="delta")
                    cb = (ite * NT + t) * P
                    nc.vector.tensor_scalar_mul(delta_t[:], ys[0][:],
                                                cnt_sb[:, cb:cb + 1])
                    for i in range(1, P):
                        nc.vector.scalar_tensor_tensor(
                            delta_t[:], ys[i][:], cnt_sb[:, cb + i:cb + i + 1],
                            delta_t[:], MULT, ADD)
                    tp = psumT.tile([128, KC * 128], f32, tag="tp")
                    if ite < NUM_ITER - 1:
                        new_updT = updTp.tile([128, KC * 128], f32r, tag="updT")
                        for k in range(KC):
                            nc.tensor.transpose(
                                tp[:, k * 128:(k + 1) * 128],
                                delta_t[:, k * 128:(k + 1) * 128], ident_sb[:])
                        for k in range(KC):
                            nc.vector.tensor_add(
                                new_updT[:, k * 128:(k + 1) * 128],
                                updT_t[:, k * 128:(k + 1) * 128].bitcast(f32),
                                tp[:, k * 128:(k + 1) * 128])
                        updT_t = new_updT
                    else:
                        out_t = outp.tile([128, D], f32, tag="out")
                        for k in range(KC):
                            nc.tensor.transpose(
                                tp[:, k * 128:(k + 1) * 128],
                                updT_t[:, k * 128:(k + 1) * 128].bitcast(f32),
                                ident_sb[:])
                        nc.vector.tensor_add(out_t[:], tp[:], delta_t[:])
                        nc.sync.dma_start(out_d[t * TP:(t + 1) * TP, :], out_t[:])
    nc.compile()
    return nc


def _prep_inputs(x, W, groups):
    cnt = np.stack([np.bincount(groups[i].ravel().astype(np.int64), minlength=N)
                    for i in range(P)]).astype(np.float32)        # [P, N]
    # cnts_host[p, (ite*NT + t)*P + i] = cnt[i, t*128+p] / (ite+1)
    cnts = np.empty((128, NUM_ITER * NT * P), dtype=np.float32)
    c_tiles = cnt.reshape(P, NT, TP)                              # [P, NT, 128]
    for ite in range(NUM_ITER):
        blk = (c_tiles / (ite + 1)).transpose(2, 1, 0)            # [128, NT, P]
        cnts[:, ite * NT * P:(ite + 1) * NT * P] = blk.reshape(128, NT * P)
    W = np.ascontiguousarray(W, dtype=np.float32)
    in_maps = []
    for b in range(B):
        xT = np.ascontiguousarray(x[b].T.astype(np.float32, copy=False))
        in_maps.append({"xT": xT, "w": W, "cnts": cnts})
    return in_maps


def kernel(x, W, groups, _trace=False, _trace_kwargs=None):
    from concourse.bass_utils import run_bass_kernel_spmd

    if "nc" not in _CACHE:
        _CACHE["nc"] = _build()
    nc = _CACHE["nc"]

    in_maps = _prep_inputs(np.asarray(x), np.asarray(W), np.asarray(groups))
    kw = {}
    if _trace:
        kw = {"trace": True, **(_trace_kwargs or {})}
    res = run_bass_kernel_spmd(nc, in_maps, core_ids=list(range(NCORES)), **kw)
    _CACHE["last_result"] = res
    out = np.stack([res.results[b]["out"] for b in range(B)]).astype(np.float32)
    return out


# revision 4
# speedup vs baseline: 1.0512x; 1.0512x over previous
"""Trainium2 Bass kernel for nn_GroupProjection (gnn_message_passing).

Reference computation (B=8, N=8192, D=512, P=4, G=512, GS=16, 3 iters):
    for ite in range(3):
        delta = 0
        for i in range(P):
            gx = upd[:, groups[i], :]                 # gather
            dx = (1/(ite+1)) * gx @ W[i]              # GEMM
            delta[:, groups[i].ravel(), :] += dx      # scatter-add
        upd = upd + delta

Key identity: the gather index equals the scatter index, so
    delta[b, n, :] = (1/(ite+1)) * sum_i count_i[n] * (upd[b, n, :] @ W[i])
where count_i[n] = multiplicity of n in groups[i]. The gather/scatter
disappears into dense GEMMs plus a per-row weighted combine; counts are
computed on host with np.bincount (groups is only 32K ints).

Row n's update depends only on row n, so the whole 3-iteration computation
streams independently over 64 row-tiles of 128 per core, data-parallel over
the batch axis (1 batch element per NeuronCore).

Device layout per row-tile (n = 128 rows):
  updT  [d=4x128 part, n=128] bf16 (x arrives host-transposed+bf16-cast, so
        DMA loads it directly; bf16 keeps LDWEIGHTS overlapped with matmuls
        -- fp32/f32r matmuls are self-loading and serialize a ~200ns weight
        load into every matmul)
  per iter:  Y_i[n,128 x e,512] = sum_k matmul(lhsT=updT[k], rhs=W[i][k])
             delta[n,d] = sum_i (count_i*scale)[n] * Y_i
               (DVE handles Y0/Y1 via per-partition tensor_scalar ops, the
                idle Scalar engine pre-scales Y2/Y3, DVE adds them in)
             updT += transpose(delta)    (PE transpose, one fused DVE add)
  final out tile = transpose(updT) + delta -> DMA out (f32).
"""

import numpy as np

B, N, D = 8, 8192, 512
P = 4
NUM_ITER = 3
TP = 128                 # rows per tile
NT = N // TP             # 64 row tiles
KC = D // 128            # 4 contraction chunks
NCORES = 8

_CACHE = {}


def _build():
    import ml_dtypes
    import concourse.bass as bass
    import concourse.tile as tile
    from concourse import bacc, mybir

    f32 = mybir.dt.float32
    bf16 = mybir.dt.bfloat16
    MULT = mybir.AluOpType.mult
    ADD = mybir.AluOpType.add

    nc = bacc.Bacc("TRN2", target_bir_lowering=False, debug=False,
                   num_devices=NCORES)

    xT_d = nc.dram_tensor("xT", [D, N], bf16, kind="ExternalInput")
    w_d = nc.dram_tensor("w", [P, D, D], bf16, kind="ExternalInput")
    c_d = nc.dram_tensor("cnts", [128, NUM_ITER * NT * P], f32,
                         kind="ExternalInput")
    out_d = nc.dram_tensor("out", [N, D], f32, kind="ExternalOutput")
    ident_d = nc.inline_tensor(np.eye(128).astype(ml_dtypes.bfloat16), "ident")

    with tile.TileContext(nc) as tc:
        with (
            tc.tile_pool(name="const", bufs=1) as constp,
            tc.tile_pool(name="updT", bufs=4) as updTp,
            tc.tile_pool(name="delta", bufs=4) as deltap,
            tc.tile_pool(name="t23", bufs=4) as t23p,
            tc.tile_pool(name="outp", bufs=3) as outp,
            tc.tile_pool(name="psumY", bufs=6, space=bass.MemorySpace.PSUM) as psumY,
            tc.tile_pool(name="psumT", bufs=2, space=bass.MemorySpace.PSUM) as psumT,
        ):
            # W: chunk (i, k) lives at columns [(i*KC+k)*D : +D]
            w_sb = constp.tile([128, P * KC * D], bf16)
            for i in range(P):
                for k in range(KC):
                    nc.sync.dma_start(
                        w_sb[:, (i * KC + k) * D:(i * KC + k + 1) * D],
                        w_d[i, k * 128:(k + 1) * 128, :])
            cnt_sb = constp.tile([128, NUM_ITER * NT * P], f32)
            nc.sync.dma_start(cnt_sb[:], c_d[:])
            ident_sb = constp.tile([128, 128], bf16)
            nc.sync.dma_start(ident_sb[:], ident_d.ap())

            for t in range(NT):
                updT_t = updTp.tile([128, KC * 128], bf16, tag="updT")
                for k in range(KC):
                    nc.sync.dma_start(
                        updT_t[:, k * 128:(k + 1) * 128],
                        xT_d[k * 128:(k + 1) * 128, t * TP:(t + 1) * TP])
                for ite in range(NUM_ITER):
                    ys = []
                    for i in range(P):
                        y = psumY.tile([128, D], f32, tag="y")
                        for k in range(KC):
                            nc.tensor.matmul(
                                y[:],
                                updT_t[:, k * 128:(k + 1) * 128],
                                w_sb[:, (i * KC + k) * D:(i * KC + k + 1) * D],
                                start=(k == 0), stop=(k == KC - 1))
                        ys.append(y)
                    # delta = sum_i (cnt_i / (ite+1)) * Y_i (scale folded into
                    # cnts). ACT pre-scales Y2/Y3; DVE does Y0/Y1 + the adds.
                    cb = (ite * NT + t) * P
                    t2 = t23p.tile([128, D], bf16, tag="t23")
                    nc.scalar.mul(t2[:], ys[2][:], cnt_sb[:, cb + 2:cb + 3])
                    t3 = t23p.tile([128, D], bf16, tag="t23")
                    nc.scalar.mul(t3[:], ys[3][:], cnt_sb[:, cb + 3:cb + 4])
                    delta_t = deltap.tile([128, D], bf16, tag="delta")
                    nc.vector.tensor_scalar_mul(delta_t[:], ys[0][:],
                                                cnt_sb[:, cb:cb + 1])
                    nc.vector.scalar_tensor_tensor(
                        delta_t[:], ys[1][:], cnt_sb[:, cb + 1:cb + 2],
                        delta_t[:], MULT, ADD)
                    nc.vector.tensor_add(delta_t[:], delta_t[:], t2[:])
                    nc.vector.tensor_add(delta_t[:], delta_t[:], t3[:])
                    tp = psumT.tile([128, KC * 128], bf16, tag="tp")
                    if ite < NUM_ITER - 1:
                        new_updT = updTp.tile([128, KC * 128], bf16, tag="updT")
                        for k in range(KC):
                            nc.tensor.transpose(
                                tp[:, k * 128:(k + 1) * 128],
                                delta_t[:, k * 128:(k + 1) * 128], ident_sb[:])
                        nc.vector.tensor_add(new_updT[:], updT_t[:], tp[:])
                        updT_t = new_updT
                    else:
                        out_t = outp.tile([128, D], f32, tag="out")
                        for k in range(KC):
                            nc.tensor.transpose(
                                tp[:, k * 128:(k + 1) * 128],
                                updT_t[:, k * 128:(k + 1) * 128], ident_sb[:])
                        nc.vector.tensor_add(out_t[:], tp[:], delta_t[:])
                        nc.sync.dma_start(out_d[t * TP:(t + 1) * TP, :], out_t[:])
    nc.compile()
    return nc


def _prep_inputs(x, W, groups):
    import ml_dtypes

    bf16 = ml_dtypes.bfloat16
    cnt = np.stack([np.bincount(groups[i].ravel().astype(np.int64), minlength=N)
                    for i in range(P)]).astype(np.float32)        # [P, N]
    # cnts_host[p, (ite*NT + t)*P + i] = cnt[i, t*128+p] / (ite+1)
    cnts = np.empty((128, NUM_ITER * NT * P), dtype=np.float32)
    c_tiles = cnt.reshape(P, NT, TP)                              # [P, NT, 128]
    for ite in range(NUM_ITER):
        blk = (c_tiles / (ite + 1)).transpose(2, 1, 0)            # [128, NT, P]
        cnts[:, ite * NT * P:(ite + 1) * NT * P] = blk.reshape(128, NT * P)
    Wb = np.ascontiguousarray(W.astype(bf16))
    in_maps = []
    for b in range(B):
        xT = np.ascontiguousarray(x[b].T.astype(bf16))
        in_maps.append({"xT": xT, "w": Wb, "cnts": cnts})
    return in_maps


def kernel(x, W, groups, _trace=False, _trace_kwargs=None):
    from concourse.bass_utils import run_bass_kernel_spmd

    if "nc" not in _CACHE:
        _CACHE["nc"] = _build()
    nc = _CACHE["nc"]

    in_maps = _prep_inputs(np.asarray(x), np.asarray(W), np.asarray(groups))
    kw = {}
    if _trace:
        kw = {"trace": True, **(_trace_kwargs or {})}
    res = run_bass_kernel_spmd(nc, in_maps, core_ids=list(range(NCORES)), **kw)
    _CACHE["last_result"] = res
    out = np.stack([res.results[b]["out"] for b in range(B)]).astype(np.float32)
    return out


# revision 6
# speedup vs baseline: 1.0522x; 1.0009x over previous
"""Trainium2 Bass kernel for nn_GroupProjection (gnn_message_passing).

Reference computation (B=8, N=8192, D=512, P=4, G=512, GS=16, 3 iters):
    for ite in range(3):
        delta = 0
        for i in range(P):
            gx = upd[:, groups[i], :]                 # gather
            dx = (1/(ite+1)) * gx @ W[i]              # GEMM
            delta[:, groups[i].ravel(), :] += dx      # scatter-add
        upd = upd + delta

Key identity: the gather index equals the scatter index, so
    delta[b, n, :] = (1/(ite+1)) * sum_i count_i[n] * (upd[b, n, :] @ W[i])
where count_i[n] = multiplicity of n in groups[i]. The gather/scatter
disappears into dense GEMMs plus a per-row weighted combine; counts are
computed on host with np.bincount (groups is only 32K ints).

Row n's update depends only on row n, so the whole 3-iteration computation
streams independently over 64 row-tiles of 128 per core, data-parallel over
the batch axis (1 batch element per NeuronCore).

Device layout per row-tile (n = 128 rows):
  updT  [d=4x128 part, n=128] bf16 (x arrives host-transposed+bf16-cast, so
        DMA loads it directly; bf16 keeps LDWEIGHTS overlapped with matmuls
        -- fp32/f32r matmuls are self-loading and serialize a ~200ns weight
        load into every matmul)
  per iter:  Y_i[n,128 x e,512] = sum_k matmul(lhsT=updT[k], rhs=W[i][k])
             delta[n,d] = sum_i (count_i*scale)[n] * Y_i
               (DVE handles Y0/Y1 via per-partition tensor_scalar ops, the
                idle Scalar engine pre-scales Y2/Y3, DVE adds them in)
             updT += transpose(delta)    (PE transpose, one fused DVE add)
  final out tile = transpose(updT) + delta -> DMA out (f32).
"""

import numpy as np

B, N, D = 8, 8192, 512
P = 4
NUM_ITER = 3
TP = 128                 # rows per tile
NT = N // TP             # 64 row tiles
KC = D // 128            # 4 contraction chunks
NCORES = 8

_CACHE = {}


def _build():
    import ml_dtypes
    import concourse.bass as bass
    import concourse.tile as tile
    from concourse import bacc, mybir

    f32 = mybir.dt.float32
    bf16 = mybir.dt.bfloat16
    MULT = mybir.AluOpType.mult
    ADD = mybir.AluOpType.add

    nc = bacc.Bacc("TRN2", target_bir_lowering=False, debug=False,
                   num_devices=NCORES)

    xT_d = nc.dram_tensor("xT", [D, N], bf16, kind="ExternalInput")
    w_d = nc.dram_tensor("w", [P, D, D], bf16, kind="ExternalInput")
    c_d = nc.dram_tensor("cnts", [128, NUM_ITER * NT * P], f32,
                         kind="ExternalInput")
    out_d = nc.dram_tensor("out", [N, D], f32, kind="ExternalOutput")
    ident_d = nc.inline_tensor(np.eye(128).astype(ml_dtypes.bfloat16), "ident")

    with tile.TileContext(nc) as tc:
        with (
            tc.tile_pool(name="const", bufs=1) as constp,
            tc.tile_pool(name="updT", bufs=4) as updTp,
            tc.tile_pool(name="delta", bufs=4) as deltap,
            tc.tile_pool(name="t23", bufs=4) as t23p,
            tc.tile_pool(name="outp", bufs=3) as outp,
            tc.tile_pool(name="psumY", bufs=7, space=bass.MemorySpace.PSUM) as psumY,
            tc.tile_pool(name="psumT", bufs=1, space=bass.MemorySpace.PSUM) as psumT,
        ):
            # W: chunk (i, k) lives at columns [(i*KC+k)*D : +D]
            w_sb = constp.tile([128, P * KC * D], bf16)
            for i in range(P):
                for k in range(KC):
                    nc.sync.dma_start(
                        w_sb[:, (i * KC + k) * D:(i * KC + k + 1) * D],
                        w_d[i, k * 128:(k + 1) * 128, :])
            cnt_sb = constp.tile([128, NUM_ITER * NT * P], f32)
            nc.sync.dma_start(cnt_sb[:], c_d[:])
            ident_sb = constp.tile([128, 128], bf16)
            nc.sync.dma_start(ident_sb[:], ident_d.ap())

            for t in range(NT):
                updT_t = updTp.tile([128, KC * 128], bf16, tag="updT")
                for k in range(KC):
                    nc.sync.dma_start(
                        updT_t[:, k * 128:(k + 1) * 128],
                        xT_d[k * 128:(k + 1) * 128, t * TP:(t + 1) * TP])
                for ite in range(NUM_ITER):
                    ys = []
                    for i in range(P):
                        y = psumY.tile([128, D], f32, tag="y")
                        for k in range(KC):
                            nc.tensor.matmul(
                                y[:],
                                updT_t[:, k * 128:(k + 1) * 128],
                                w_sb[:, (i * KC + k) * D:(i * KC + k + 1) * D],
                                start=(k == 0), stop=(k == KC - 1))
                        ys.append(y)
                    # delta = sum_i (cnt_i / (ite+1)) * Y_i (scale folded into
                    # cnts). ACT pre-scales Y2/Y3; DVE does Y0/Y1 + the adds.
                    cb = (ite * NT + t) * P
                    t0 = t23p.tile([128, D], bf16, tag="t23")
                    nc.scalar.mul(t0[:], ys[0][:], cnt_sb[:, cb:cb + 1])
                    t1 = t23p.tile([128, D], bf16, tag="t23")
                    nc.scalar.mul(t1[:], ys[1][:], cnt_sb[:, cb + 1:cb + 2])
                    delta_t = deltap.tile([128, D], bf16, tag="delta")
                    nc.vector.tensor_scalar_mul(delta_t[:], ys[2][:],
                                                cnt_sb[:, cb + 2:cb + 3])
                    nc.vector.scalar_tensor_tensor(
                        delta_t[:], ys[3][:], cnt_sb[:, cb + 3:cb + 4],
                        delta_t[:], MULT, ADD)
                    nc.vector.tensor_add(delta_t[:], delta_t[:], t0[:])
                    nc.vector.tensor_add(delta_t[:], delta_t[:], t1[:])
                    tp = psumT.tile([128, KC * 128], bf16, tag="tp")
                    if ite < NUM_ITER - 1:
                        new_updT = updTp.tile([128, KC * 128], bf16, tag="updT")
                        for k in range(KC):
                            nc.tensor.transpose(
                                tp[:, k * 128:(k + 1) * 128],
                                delta_t[:, k * 128:(k + 1) * 128], ident_sb[:])
                        nc.vector.tensor_add(new_updT[:], updT_t[:], tp[:])
                        updT_t = new_updT
                    else:
                        out_t = outp.tile([128, D], f32, tag="out")
                        for k in range(KC):
                            nc.tensor.transpose(
                                tp[:, k * 128:(k + 1) * 128],
                                updT_t[:, k * 128:(k + 1) * 128], ident_sb[:])
                        nc.vector.tensor_add(out_t[:], tp[:], delta_t[:])
                        nc.sync.dma_start(out_d[t * TP:(t + 1) * TP, :], out_t[:])
    nc.compile()
    return nc


def _prep_inputs(x, W, groups):
    import ml_dtypes

    bf16 = ml_dtypes.bfloat16
    cnt = np.stack([np.bincount(groups[i].ravel().astype(np.int64), minlength=N)
                    for i in range(P)]).astype(np.float32)        # [P, N]
    # cnts_host[p, (ite*NT + t)*P + i] = cnt[i, t*128+p] / (ite+1)
    cnts = np.empty((128, NUM_ITER * NT * P), dtype=np.float32)
    c_tiles = cnt.reshape(P, NT, TP)                              # [P, NT, 128]
    for ite in range(NUM_ITER):
        blk = (c_tiles / (ite + 1)).transpose(2, 1, 0)            # [128, NT, P]
        cnts[:, ite * NT * P:(ite + 1) * NT * P] = blk.reshape(128, NT * P)
    Wb = np.ascontiguousarray(W.astype(bf16))
    in_maps = []
    for b in range(B):
        xT = np.ascontiguousarray(x[b].T.astype(bf16))
        in_maps.append({"xT": xT, "w": Wb, "cnts": cnts})
    return in_maps


def kernel(x, W, groups, _trace=False, _trace_kwargs=None):
    from concourse.bass_utils import run_bass_kernel_spmd

    if "nc" not in _CACHE:
        _CACHE["nc"] = _build()
    nc = _CACHE["nc"]

    in_maps = _prep_inputs(np.asarray(x), np.asarray(W), np.asarray(groups))
    kw = {}
    if _trace:
        kw = {"trace": True, **(_trace_kwargs or {})}
    res = run_bass_kernel_spmd(nc, in_maps, core_ids=list(range(NCORES)), **kw)
    _CACHE["last_result"] = res
    out = np.stack([res.results[b]["out"] for b in range(B)]).astype(np.float32)
    return out
